# revision 45
# baseline (speedup 1.0000x reference)
"""Trainium2 Bass kernel for the NumReps masked-mean problem.

Math: each mask row is a contiguous run of ones (1..8 long). expand_window
widens it by int(0.2*len) (== 1 iff len >= 5) on each side, clamped to
[0, S-1]; the output row is the mean of reps rows over the widened window
(window length n <= 10, and n is never 5 or 6).

Schedule (per core, data-parallel over batch: 16 batches / 8 cores = 2;
baseline 56.2us -> 49.1us on HW):
  - both masks stream first on the sync HWDGE queue (contiguous 1MB
    transfers, FIFO so batch 0 lands first); the possum iota row streams
    concurrently on the scalar HWDGE queue from an inline const
  - conditional-gather landing area pre-zeroed at t0, split across
    vector/scalar/gpsimd
  - per batch: len via scalar ACT accum in parallel with the position-sum
    (DVE STT accum); a short DVE chain recovers the exact window start
    (rint trick) and triggers two indirect-DMA gathers: rows ns..ns+3
    always, rows ns+4..ns+9 only where n >= 7 (OOB index -> skipped).
    Batch 1's possum takes its "-4096" from a tile computed off batch 0's
    indices -- a data dep that stops the Tile scheduler from hoisting it
    ahead of batch 0's chain (it otherwise delays the first gather 2us)
  - weighted windowed sum on the TensorEngine: the verifier-forced
    f32 -> f32r staging pass doubles as the weight multiply (per-j
    tensor_scalar), so the 10 accumulating matmuls per 512-col half use
    one constant f32r identity as lhsT; scale/matmul order [4..9, 0..3]
    measured fastest (6us better than [0..9])
  - PSUM -> SBUF copies split across vector/scalar, two half out-DMAs
"""

import numpy as np

B, M, S, D = 16, 128, 2048, 1024
NCORES = 8
BPC = B // NCORES  # batches per core
WMAX = 10  # max expanded window length
RINT_MAGIC = 12582912.0  # 2^23 + 2^22: (x + magic) - magic == rint(x)

# gather chunks (row_offset, n_rows, skip-threshold on nm = n-1), in trigger
# order. n is in {1,2,3,4,7,8,9,10}: rows 0-3 always (n<=4 windows fit),
# rows 4-9 iff n>=7. Finer splits save HBM bytes but lose more to the
# ~1.2us/chunk serial INDIRECT1D descriptor-gen on gpsimd (measured: a
# 4-chunk scheme regressed 1.4us). The unconditional head triggers LAST so
# the final landing bytes only gate 8 matmuls.
GCHUNKS = [(4, 6, 5.5), (0, 4, None)]
ZBASE = 4  # first gather column that can be skipped -> pre-zeroed from here

# The walrus BIR verifier (checkMatmultFP32r) requires every f32r matmul
# input's producer instruction to be a rounding op, so the DMA-written
# gather tile cannot feed the PE directly. The window weights are folded
# into that forced staging pass (tensor_scalar mult, f32 -> f32r); the
# lhsT then degenerates to a constant identity and no diag build is
# needed. Scaling engines: 8 chunks on DVE, 2 on scalar ACT.

_cache = {}


def _build_nc():
    import concourse.bacc as bacc
    import concourse.bass as bass
    import concourse.mybir as mybir
    from concourse import tile

    f32 = mybir.dt.float32
    f32r = mybir.dt.float32r
    i32 = mybir.dt.int32
    i16 = mybir.dt.int16
    i64 = mybir.dt.int64
    Alu = mybir.AluOpType
    Act = mybir.ActivationFunctionType

    nc = bacc.Bacc("TRN2", target_bir_lowering=False, debug=False)

    iota_np = np.broadcast_to(np.arange(S, dtype=np.int16), (M, S))
    iota_const = nc.inline_tensor(np.ascontiguousarray(iota_np),
                                  name="iota_const")

    mask = nc.dram_tensor("mask", [BPC, M, S], f32, kind="ExternalInput")
    reps = [
        nc.dram_tensor(f"reps{b}", [S, D], f32, kind="ExternalInput")
        for b in range(BPC)
    ]
    out = nc.dram_tensor("out", [BPC, M, D], f32, kind="ExternalOutput")

    with tile.TileContext(nc) as tc:
        with (
            tc.tile_pool(name="const", bufs=1) as cpool,
            tc.tile_pool(name="big", bufs=2) as big,
            tc.tile_pool(name="small", bufs=2) as small,
            tc.tile_pool(name="psum", bufs=2, space="PSUM") as psum,
        ):
            # ---- mask DMAs first: both on the sync HWDGE queue (FIFO), so
            # batch 0's mask streams first at full rate and batch 1 follows
            mts = {}
            for b in range(BPC):
                mt = cpool.tile([M, S], f32, tag=f"mt{b}", name=f"mt{b}")
                nc.sync.dma_start(mt[:], mask[b])
                mts[b] = mt

            # ---- t0 block: constants + tail pre-zero, all off the mask path
            iota_w = cpool.tile([M, WMAX], i16)
            nc.gpsimd.iota(iota_w[:], pattern=[[1, WMAX]], base=0,
                           channel_multiplier=0)
            # iota row for possum: DMA'd from an inline const on the scalar
            # HWDGE queue (streams during startup; keeps gpsimd free for the
            # gather descriptor generation)
            iota_s = cpool.tile([M, S], i16)
            nc.scalar.dma_start(iota_s[:], iota_const[:])
            ident = cpool.tile([M, M], f32)
            nc.gpsimd.memset(ident[:], 1.0)
            nc.gpsimd.affine_select(
                out=ident[:], in_=ident[:], compare_op=Alu.is_equal,
                fill=0.0, base=0, pattern=[[-1, M]], channel_multiplier=1,
            )
            # f32r copy of the identity (DVE = rounding op, verifier-legal)
            ident_r = cpool.tile([M, M], f32r)
            nc.vector.tensor_copy(ident_r[:], ident[:])

            gts = [
                cpool.tile([M, WMAX * D], f32, tag=f"gt{b}", name=f"gt{b}")
                for b in range(BPC)
            ]
            # conditional-gather landing area zeroed once, split across
            # engines; int64 bitcast halves the element count (memset rate
            # is element-bound at ~150G elem/s)
            ZMID = (ZBASE + WMAX) // 2
            nc.vector.memzero(gts[0][:, ZBASE * D:ZMID * D].bitcast(i64))
            nc.scalar.memzero(gts[0][:, ZMID * D:].bitcast(i64))
            nc.gpsimd.memzero(gts[1][:, ZBASE * D:ZMID * D].bitcast(i64))
            nc.vector.memzero(gts[1][:, ZMID * D:].bitcast(i64))

            # ---- per-batch: lengths, indices, gathers
            ws = {}
            prev_ind = None
            for b in range(BPC):
                # len on scalar engine (accum), possum on DVE
                lsink = small.tile([M, 1], f32, tag="lsink")
                lsink_ap = bass.AP(
                    lsink[:].tensor, lsink[:].offset,
                    [lsink[:].ap[0], [0, S]],
                )
                lenf = small.tile([M, 1], f32, tag="lenf")
                nc.scalar.activation(
                    out=lsink_ap, in_=mts[b][:], func=Act.Identity,
                    accum_out=lenf[:],
                )
                ssink = small.tile([M, 1], f32, tag="ssink")
                ssink_ap = bass.AP(
                    ssink[:].tensor, ssink[:].offset,
                    [ssink[:].ap[0], [0, S]],
                )
                # b1's possum uses a computed "-4096" that depends on b0's
                # gather indices: a data dep that stops the Tile scheduler
                # from hoisting this 2.3us DVE pass ahead of b0's index
                # chain (which would delay b0's gather trigger). gpsimd
                # cannot run scalar_tensor_tensor (walrus engine check), so
                # both passes stay on DVE.
                poss_scalar = -4096.0 if b == 0 else fake_m4096[:, :1]
                a1 = small.tile([M, 1], f32, tag="a1")
                nc.vector.scalar_tensor_tensor(
                    out=ssink_ap, in0=iota_s[:], scalar=poss_scalar,
                    in1=mts[b][:], op0=Alu.add, op1=Alu.mult,
                    accum_out=a1[:],
                )

                # index chain (DVE), gather indices first
                rl = small.tile([M, 1], f32, tag="rl")
                nc.vector.reciprocal(rl[:], lenf[:])
                psm = small.tile([M, 1], f32, tag="psm")
                nc.vector.tensor_scalar(
                    out=psm[:], in0=lenf[:], scalar1=4096.0,
                    scalar2=a1[:, :1], op0=Alu.mult, op1=Alu.add,
                )
                hl = small.tile([M, 1], f32, tag="hl")
                nc.vector.tensor_scalar(
                    out=hl[:], in0=lenf[:], scalar1=-1.0, scalar2=0.5,
                    op0=Alu.add, op1=Alu.mult,
                )
                first = small.tile([M, 1], f32, tag="first")
                nc.vector.tensor_scalar(
                    out=first[:], in0=psm[:], scalar1=rl[:, :1],
                    scalar2=hl[:, :1], op0=Alu.mult, op1=Alu.subtract,
                )
                nc.vector.tensor_scalar(
                    out=first[:], in0=first[:], scalar1=RINT_MAGIC,
                    scalar2=-RINT_MAGIC, op0=Alu.add, op1=Alu.add,
                )
                # em1 = e - 1 = (len >= 5) - 1
                em1 = small.tile([M, 1], f32, tag="em1")
                nc.vector.tensor_scalar(
                    out=em1[:], in0=lenf[:], scalar1=4.5, scalar2=-1.0,
                    op0=Alu.is_ge, op1=Alu.add,
                )
                # ns = max(first - e, 0) = max(first - em1 - 1, 0)
                ns = small.tile([M, 1], f32, tag="ns")
                nc.vector.tensor_scalar(
                    out=ns[:], in0=first[:], scalar1=em1[:, :1],
                    scalar2=-1.0, op0=Alu.subtract, op1=Alu.add,
                )
                nc.vector.tensor_scalar_max(ns[:], ns[:], 0.0)
                # tail chunk (rows 4-9) fires iff n >= 7 iff e == 1:
                # idx = ns + 4 - 4096*em1 (e=0 -> pushed past the bounds
                # check and silently dropped, landing area pre-zeroed).
                # Computed BEFORE the head index so the tail triggers first
                # and the head (closing matmuls j0..3) lands last.
                idx2 = small.tile([M, 1], f32, tag="idx2")
                nc.vector.tensor_scalar(
                    out=idx2[:], in0=em1[:], scalar1=-4096.0,
                    scalar2=ns[:, :1], op0=Alu.mult, op1=Alu.add,
                )
                nc.vector.tensor_scalar_add(idx2[:], idx2[:], float(ZBASE))
                nsi2 = small.tile([M, 1], i32, tag="nsi2")
                nc.vector.tensor_copy(nsi2[:], idx2[:])
                nsi = small.tile([M, 1], i32, tag="nsi")
                nc.vector.tensor_copy(nsi[:], ns[:])
                if b == 0:
                    fake_m4096 = small.tile([M, 1], f32, tag="fake")
                    nc.vector.tensor_scalar(
                        out=fake_m4096[:], in0=idx2[:], scalar1=0.0,
                        scalar2=-4096.0, op0=Alu.mult, op1=Alu.add,
                    )
                ind_t = nc.gpsimd.indirect_dma_start(
                    out=gts[b][:, ZBASE * D:],
                    out_offset=None,
                    in_=reps[b][:],
                    in_offset=bass.IndirectOffsetOnAxis(ap=nsi2[:, :1],
                                                        axis=0),
                    bounds_check=S - 1,
                    oob_is_err=False,
                )
                ind_h = nc.gpsimd.indirect_dma_start(
                    out=gts[b][:, :ZBASE * D],
                    out_offset=None,
                    in_=reps[b][:],
                    in_offset=bass.IndirectOffsetOnAxis(ap=nsi[:, :1],
                                                        axis=0),
                )
                # NOTE: the Tile scheduler reorders the two triggers (no
                # data dep) to head-first on the wire; pinning tail-first
                # with nosync edges measured 3.5us SLOWER, so leave it be
                del ind_t, ind_h

                # n, inv, weights (DVE, off the gather-trigger path)
                nep = small.tile([M, 1], f32, tag="nep")
                nc.vector.tensor_scalar(
                    out=nep[:], in0=first[:], scalar1=lenf[:, :1],
                    scalar2=em1[:, :1], op0=Alu.add, op1=Alu.add,
                )
                nm = small.tile([M, 1], f32, tag="nm")  # n - 1 = ne - ns
                nc.vector.tensor_scalar(
                    out=nm[:], in0=nep[:], scalar1=float(S - 1),
                    scalar2=ns[:, :1], op0=Alu.min, op1=Alu.subtract,
                )
                n = small.tile([M, 1], f32, tag="n")
                nc.vector.tensor_scalar_add(n[:], nm[:], 1.0)
                inv = small.tile([M, 1], f32, tag="inv")
                nc.vector.reciprocal(inv[:], n[:])
                w = small.tile([M, WMAX], f32, tag="w")
                nc.vector.tensor_scalar(
                    out=w[:], in0=iota_w[:, :WMAX], scalar1=nm[:, :1],
                    scalar2=inv[:, :1], op0=Alu.is_le, op1=Alu.mult,
                )
                ws[b] = w

            # ---- per-batch: weight-fold staging + identity matmuls + store
            for b in range(BPC):
                gt = gts[b]
                w = ws[b]

                # j order: tail chunks 4..9 then head 0..3 (PSUM
                # accumulation is order-agnostic). Empirically the fastest
                # order (plain 0..9 measured 6us slower); the final two
                # chunks' scales split across DVE/scalar to run in parallel.
                jorder = [j for off, nrows, _ in GCHUNKS
                          for j in range(off, off + nrows)]
                segs = {}
                for idx, j in enumerate(jorder):
                    gsc = big.tile([M, D], f32r, tag="gsc", bufs=8,
                                   name=f"gsc_{b}_{j}")
                    if idx not in (7, 9):
                        nc.vector.tensor_scalar(
                            out=gsc[:], in0=gt[:, j * D:(j + 1) * D],
                            scalar1=w[:, j:j + 1], scalar2=None, op0=Alu.mult,
                        )
                    else:
                        nc.scalar.activation(
                            out=gsc[:], in_=gt[:, j * D:(j + 1) * D],
                            func=Act.Identity, scale=w[:, j:j + 1],
                        )
                    segs[j] = gsc

                ps0 = psum.tile([M, 512], f32, tag="ps0")
                ps1 = psum.tile([M, 512], f32, tag="ps1")
                for idx, j in enumerate(jorder):
                    nc.tensor.matmul(
                        ps0[:], lhsT=ident_r[:], rhs=segs[j][:, :512],
                        start=(idx == 0), stop=(idx == WMAX - 1),
                    )
                    nc.tensor.matmul(
                        ps1[:], lhsT=ident_r[:], rhs=segs[j][:, 512:],
                        start=(idx == 0), stop=(idx == WMAX - 1),
                    )

                osum = big.tile([M, D], f32, tag="osum")
                if b == 0:
                    nc.scalar.copy(osum[:, :512], ps0[:])
                    nc.scalar.copy(osum[:, 512:], ps1[:])
                else:
                    nc.vector.tensor_copy(osum[:, :512], ps0[:])
                    nc.scalar.copy(osum[:, 512:], ps1[:])
                # halves on separate HWDGE queues (sync / scalar ring rows)
                # so the transfers stream concurrently instead of FIFO
                nc.sync.dma_start(out[b][:, :512], osum[:, :512])
                nc.scalar.dma_start(out[b][:, 512:], osum[:, 512:])

    nc.finalize()
    return nc


def _get_nc():
    if "nc" not in _cache:
        _cache["nc"] = _build_nc()
    return _cache["nc"]


def _shard_inputs(number_mask, reps):
    in_maps = []
    for c in range(NCORES):
        m = {"mask": np.ascontiguousarray(number_mask[c * BPC:(c + 1) * BPC])}
        for b in range(BPC):
            m[f"reps{b}"] = np.ascontiguousarray(reps[c * BPC + b])
        in_maps.append(m)
    return in_maps


def _install_ntff_hook():
    """The image's antenv lacks axon_hooks; synthesize it so trace=True
    (NTFF profiling) works through run_bass_kernel_spmd."""
    import sys
    import types

    try:
        from antenv.axon_hooks import get_axon_ntff_profile_hook  # noqa: F401
        return
    except ImportError:
        pass
    from trn_agent_boot.trn_boot import _ntff_profile_via_ctypes

    mod = types.ModuleType("antenv.axon_hooks")
    _hook = [_ntff_profile_via_ctypes("/opt/axon/libaxon_pjrt.so")]
    mod.get_axon_ntff_profile_hook = lambda: _hook[0]
    mod.set_axon_ntff_profile_hook = lambda h: _hook.__setitem__(0, h)
    sys.modules["antenv.axon_hooks"] = mod
    import antenv

    antenv.axon_hooks = mod


def _run(number_mask, reps, trace=False):
    from concourse.bass_utils import run_bass_kernel_spmd

    if trace:
        _install_ntff_hook()
    nc = _get_nc()
    in_maps = _shard_inputs(number_mask, reps)
    res = run_bass_kernel_spmd(
        nc, in_maps, core_ids=list(range(NCORES)), trace=trace
    )
    outs = np.stack([r["out"] for r in res.results], axis=0)
    return outs.reshape(B, M, D), res


def kernel(**inputs):
    out, _ = _run(inputs["number_mask"], inputs["reps"], trace=False)
    return out


# revision 46
# speedup vs baseline: 1.0470x; 1.0470x over previous
"""Trainium2 Bass kernel for the NumReps masked-mean problem.

Math: each mask row is a contiguous run of ones (1..8 long). expand_window
widens it by int(0.2*len) (== 1 iff len >= 5) on each side, clamped to
[0, S-1]; the output row is the mean of reps rows over the widened window
(window length n <= 10, and n is never 5 or 6).

Schedule (per core, data-parallel over batch: 16 batches / 8 cores = 2;
baseline 56.2us -> 49.1us on HW):
  - both masks stream first on the sync HWDGE queue (contiguous 1MB
    transfers, FIFO so batch 0 lands first); the possum iota row streams
    concurrently on the scalar HWDGE queue from an inline const
  - conditional-gather landing area pre-zeroed at t0, split across
    vector/scalar/gpsimd
  - per batch: len via scalar ACT accum in parallel with the position-sum
    (DVE STT accum); a short DVE chain recovers the exact window start
    (rint trick) and triggers two indirect-DMA gathers: rows ns..ns+3
    always, rows ns+4..ns+9 only where n >= 7 (OOB index -> skipped).
    Batch 1's possum takes its "-4096" from a tile computed off batch 0's
    indices -- a data dep that stops the Tile scheduler from hoisting it
    ahead of batch 0's chain (it otherwise delays the first gather 2us)
  - weighted windowed sum on the TensorEngine: the verifier-forced
    f32 -> f32r staging pass doubles as the weight multiply (per-j
    tensor_scalar), so the 10 accumulating matmuls per 512-col half use
    one constant f32r identity as lhsT; scale/matmul order [4..9, 0..3]
    measured fastest (6us better than [0..9])
  - PSUM -> SBUF copies split across vector/scalar, two half out-DMAs
"""

import numpy as np

B, M, S, D = 16, 128, 2048, 1024
NCORES = 8
BPC = B // NCORES  # batches per core
WMAX = 10  # max expanded window length
RINT_MAGIC = 12582912.0  # 2^23 + 2^22: (x + magic) - magic == rint(x)

# gather chunks (row_offset, n_rows, skip-threshold on nm = n-1), in trigger
# order. n is in {1,2,3,4,7,8,9,10}: rows 0-3 always (n<=4 windows fit),
# rows 4-9 iff n>=7. Finer splits save HBM bytes but lose more to the
# ~1.2us/chunk serial INDIRECT1D descriptor-gen on gpsimd (measured: a
# 4-chunk scheme regressed 1.4us). The unconditional head triggers LAST so
# the final landing bytes only gate 8 matmuls.
GCHUNKS = [(4, 6, 5.5), (0, 4, None)]
ZBASE = 4  # first gather column that can be skipped -> pre-zeroed from here

# The walrus BIR verifier (checkMatmultFP32r) requires every f32r matmul
# input's producer instruction to be a rounding op, so the DMA-written
# gather tile cannot feed the PE directly. The window weights are folded
# into that forced staging pass (tensor_scalar mult, f32 -> f32r); the
# lhsT then degenerates to a constant identity and no diag build is
# needed. Scaling engines: 8 chunks on DVE, 2 on scalar ACT.

_cache = {}


def _build_nc():
    import concourse.bacc as bacc
    import concourse.bass as bass
    import concourse.mybir as mybir
    from concourse import tile

    f32 = mybir.dt.float32
    f32r = mybir.dt.float32r
    i32 = mybir.dt.int32
    i16 = mybir.dt.int16
    i64 = mybir.dt.int64
    Alu = mybir.AluOpType
    Act = mybir.ActivationFunctionType

    nc = bacc.Bacc("TRN2", target_bir_lowering=False, debug=False)

    iota_np = np.broadcast_to(np.arange(S, dtype=np.int16), (M, S))
    iota_const = nc.inline_tensor(np.ascontiguousarray(iota_np),
                                  name="iota_const")

    mask = nc.dram_tensor("mask", [BPC, M, S], f32, kind="ExternalInput")
    reps = [
        nc.dram_tensor(f"reps{b}", [S, D], f32, kind="ExternalInput")
        for b in range(BPC)
    ]
    out = nc.dram_tensor("out", [BPC, M, D], f32, kind="ExternalOutput")

    with tile.TileContext(nc) as tc:
        with (
            tc.tile_pool(name="const", bufs=1) as cpool,
            tc.tile_pool(name="big", bufs=2) as big,
            tc.tile_pool(name="small", bufs=2) as small,
            tc.tile_pool(name="psum", bufs=2, space="PSUM") as psum,
        ):
            # ---- mask DMAs first: both on the sync HWDGE queue (FIFO), so
            # batch 0's mask streams first at full rate and batch 1 follows
            mts = {}
            for b in range(BPC):
                mt = cpool.tile([M, S], f32, tag=f"mt{b}", name=f"mt{b}")
                nc.sync.dma_start(mt[:], mask[b])
                mts[b] = mt

            # ---- t0 block: constants + tail pre-zero, all off the mask path
            iota_w = cpool.tile([M, WMAX], i16)
            nc.gpsimd.iota(iota_w[:], pattern=[[1, WMAX]], base=0,
                           channel_multiplier=0)
            # iota row for possum: DMA'd from an inline const on the scalar
            # HWDGE queue (streams during startup; keeps gpsimd free for the
            # gather descriptor generation)
            iota_s = cpool.tile([M, S], i16)
            nc.scalar.dma_start(iota_s[:], iota_const[:])
            ident = cpool.tile([M, M], f32)
            nc.gpsimd.memset(ident[:], 1.0)
            nc.gpsimd.affine_select(
                out=ident[:], in_=ident[:], compare_op=Alu.is_equal,
                fill=0.0, base=0, pattern=[[-1, M]], channel_multiplier=1,
            )
            # f32r copy of the identity (DVE = rounding op, verifier-legal)
            ident_r = cpool.tile([M, M], f32r)
            nc.vector.tensor_copy(ident_r[:], ident[:])

            gts = [
                cpool.tile([M, WMAX * D], f32, tag=f"gt{b}", name=f"gt{b}")
                for b in range(BPC)
            ]
            # conditional-gather landing area zeroed once, split across
            # engines; int64 bitcast halves the element count (memset rate
            # is element-bound at ~150G elem/s)
            ZMID = (ZBASE + WMAX) // 2
            nc.vector.memzero(gts[0][:, ZBASE * D:ZMID * D].bitcast(i64))
            nc.scalar.memzero(gts[0][:, ZMID * D:].bitcast(i64))
            nc.gpsimd.memzero(gts[1][:, ZBASE * D:ZMID * D].bitcast(i64))
            nc.vector.memzero(gts[1][:, ZMID * D:].bitcast(i64))

            # ---- per-batch: lengths, indices, gathers
            ws = {}
            prev_ind = None
            for b in range(BPC):
                # len on scalar engine (accum), possum on DVE
                lsink = small.tile([M, 1], f32, tag="lsink")
                lsink_ap = bass.AP(
                    lsink[:].tensor, lsink[:].offset,
                    [lsink[:].ap[0], [0, S]],
                )
                lenf = small.tile([M, 1], f32, tag="lenf")
                nc.scalar.activation(
                    out=lsink_ap, in_=mts[b][:], func=Act.Identity,
                    accum_out=lenf[:],
                )
                ssink = small.tile([M, 1], f32, tag="ssink")
                ssink_ap = bass.AP(
                    ssink[:].tensor, ssink[:].offset,
                    [ssink[:].ap[0], [0, S]],
                )
                # b1's possum uses a computed "-4096" that depends on b0's
                # gather indices: a data dep that stops the Tile scheduler
                # from hoisting this 2.3us DVE pass ahead of b0's index
                # chain (which would delay b0's gather trigger). gpsimd
                # cannot run scalar_tensor_tensor (walrus engine check), so
                # both passes stay on DVE.
                poss_scalar = -4096.0 if b == 0 else fake_m4096[:, :1]
                a1 = small.tile([M, 1], f32, tag="a1")
                nc.vector.scalar_tensor_tensor(
                    out=ssink_ap, in0=iota_s[:], scalar=poss_scalar,
                    in1=mts[b][:], op0=Alu.add, op1=Alu.mult,
                    accum_out=a1[:],
                )

                # index chain (DVE), gather indices first
                rl = small.tile([M, 1], f32, tag="rl")
                nc.vector.reciprocal(rl[:], lenf[:])
                psm = small.tile([M, 1], f32, tag="psm")
                nc.vector.tensor_scalar(
                    out=psm[:], in0=lenf[:], scalar1=4096.0,
                    scalar2=a1[:, :1], op0=Alu.mult, op1=Alu.add,
                )
                hl = small.tile([M, 1], f32, tag="hl")
                nc.vector.tensor_scalar(
                    out=hl[:], in0=lenf[:], scalar1=-1.0, scalar2=0.5,
                    op0=Alu.add, op1=Alu.mult,
                )
                first = small.tile([M, 1], f32, tag="first")
                nc.vector.tensor_scalar(
                    out=first[:], in0=psm[:], scalar1=rl[:, :1],
                    scalar2=hl[:, :1], op0=Alu.mult, op1=Alu.subtract,
                )
                nc.vector.tensor_scalar(
                    out=first[:], in0=first[:], scalar1=RINT_MAGIC,
                    scalar2=-RINT_MAGIC, op0=Alu.add, op1=Alu.add,
                )
                # em1 = e - 1 = (len >= 5) - 1
                em1 = small.tile([M, 1], f32, tag="em1")
                nc.vector.tensor_scalar(
                    out=em1[:], in0=lenf[:], scalar1=4.5, scalar2=-1.0,
                    op0=Alu.is_ge, op1=Alu.add,
                )
                # ns = max(first - e, 0) = max(first - em1 - 1, 0)
                ns = small.tile([M, 1], f32, tag="ns")
                nc.vector.tensor_scalar(
                    out=ns[:], in0=first[:], scalar1=em1[:, :1],
                    scalar2=-1.0, op0=Alu.subtract, op1=Alu.add,
                )
                nc.vector.tensor_scalar_max(ns[:], ns[:], 0.0)
                # tail chunk (rows 4-9) fires iff n >= 7 iff e == 1:
                # idx = ns + 4 - 4096*em1 (e=0 -> pushed past the bounds
                # check and silently dropped, landing area pre-zeroed).
                # Computed BEFORE the head index so the tail triggers first
                # and the head (closing matmuls j0..3) lands last.
                idx2 = small.tile([M, 1], f32, tag="idx2")
                nc.vector.tensor_scalar(
                    out=idx2[:], in0=em1[:], scalar1=-4096.0,
                    scalar2=ns[:, :1], op0=Alu.mult, op1=Alu.add,
                )
                nc.vector.tensor_scalar_add(idx2[:], idx2[:], float(ZBASE))
                nsi2 = small.tile([M, 1], i32, tag="nsi2")
                nc.vector.tensor_copy(nsi2[:], idx2[:])
                nsi = small.tile([M, 1], i32, tag="nsi")
                nc.vector.tensor_copy(nsi[:], ns[:])
                if b == 0:
                    fake_m4096 = small.tile([M, 1], f32, tag="fake")
                    nc.vector.tensor_scalar(
                        out=fake_m4096[:], in0=idx2[:], scalar1=0.0,
                        scalar2=-4096.0, op0=Alu.mult, op1=Alu.add,
                    )
                ind_t = nc.gpsimd.indirect_dma_start(
                    out=gts[b][:, ZBASE * D:],
                    out_offset=None,
                    in_=reps[b][:],
                    in_offset=bass.IndirectOffsetOnAxis(ap=nsi2[:, :1],
                                                        axis=0),
                    bounds_check=S - 1,
                    oob_is_err=False,
                )
                ind_h = nc.gpsimd.indirect_dma_start(
                    out=gts[b][:, :ZBASE * D],
                    out_offset=None,
                    in_=reps[b][:],
                    in_offset=bass.IndirectOffsetOnAxis(ap=nsi[:, :1],
                                                        axis=0),
                )
                # NOTE: the Tile scheduler reorders the two triggers (no
                # data dep) to head-first on the wire; pinning tail-first
                # with nosync edges measured 3.5us SLOWER, so leave it be
                del ind_t, ind_h

                # n, inv, weights (DVE, off the gather-trigger path)
                nep = small.tile([M, 1], f32, tag="nep")
                nc.vector.tensor_scalar(
                    out=nep[:], in0=first[:], scalar1=lenf[:, :1],
                    scalar2=em1[:, :1], op0=Alu.add, op1=Alu.add,
                )
                nm = small.tile([M, 1], f32, tag="nm")  # n - 1 = ne - ns
                nc.vector.tensor_scalar(
                    out=nm[:], in0=nep[:], scalar1=float(S - 1),
                    scalar2=ns[:, :1], op0=Alu.min, op1=Alu.subtract,
                )
                n = small.tile([M, 1], f32, tag="n")
                nc.vector.tensor_scalar_add(n[:], nm[:], 1.0)
                inv = small.tile([M, 1], f32, tag="inv")
                nc.vector.reciprocal(inv[:], n[:])
                w = small.tile([M, WMAX], f32, tag="w")
                nc.vector.tensor_scalar(
                    out=w[:], in0=iota_w[:, :WMAX], scalar1=nm[:, :1],
                    scalar2=inv[:, :1], op0=Alu.is_le, op1=Alu.mult,
                )
                ws[b] = w

            # ---- per-batch: weight-fold staging + identity matmuls + store
            for b in range(BPC):
                gt = gts[b]
                w = ws[b]

                # j order: tail chunks 4..9 then head 0..3 (PSUM
                # accumulation is order-agnostic). Empirically the fastest
                # order (plain 0..9 measured 6us slower); the final two
                # chunks' scales split across DVE/scalar to run in parallel.
                jorder = [j for off, nrows, _ in GCHUNKS
                          for j in range(off, off + nrows)]
                segs = {}
                for idx, j in enumerate(jorder):
                    gsc = big.tile([M, D], f32r, tag="gsc", bufs=8,
                                   name=f"gsc_{b}_{j}")
                    if idx not in (7, 9):
                        nc.vector.tensor_scalar(
                            out=gsc[:], in0=gt[:, j * D:(j + 1) * D],
                            scalar1=w[:, j:j + 1], scalar2=None, op0=Alu.mult,
                        )
                    else:
                        nc.scalar.activation(
                            out=gsc[:], in_=gt[:, j * D:(j + 1) * D],
                            func=Act.Identity, scale=w[:, j:j + 1],
                        )
                    segs[j] = gsc

                ps0 = psum.tile([M, 512], f32, tag="ps0")
                ps1 = psum.tile([M, 512], f32, tag="ps1")
                for idx, j in enumerate(jorder):
                    nc.tensor.matmul(
                        ps0[:], lhsT=ident_r[:], rhs=segs[j][:, :512],
                        start=(idx == 0), stop=(idx == WMAX - 1),
                    )
                    nc.tensor.matmul(
                        ps1[:], lhsT=ident_r[:], rhs=segs[j][:, 512:],
                        start=(idx == 0), stop=(idx == WMAX - 1),
                    )

                osum = big.tile([M, D], f32, tag="osum")
                if b == 0:
                    nc.scalar.copy(osum[:, :512], ps0[:])
                    nc.scalar.copy(osum[:, 512:], ps1[:])
                else:
                    nc.vector.tensor_copy(osum[:, :512], ps0[:])
                    nc.scalar.copy(osum[:, 512:], ps1[:])
                nc.sync.dma_start(out[b][:, :512], osum[:, :512])
                nc.sync.dma_start(out[b][:, 512:], osum[:, 512:])

    nc.finalize()
    return nc


def _get_nc():
    if "nc" not in _cache:
        _cache["nc"] = _build_nc()
    return _cache["nc"]


def _shard_inputs(number_mask, reps):
    in_maps = []
    for c in range(NCORES):
        m = {"mask": np.ascontiguousarray(number_mask[c * BPC:(c + 1) * BPC])}
        for b in range(BPC):
            m[f"reps{b}"] = np.ascontiguousarray(reps[c * BPC + b])
        in_maps.append(m)
    return in_maps


def _install_ntff_hook():
    """The image's antenv lacks axon_hooks; synthesize it so trace=True
    (NTFF profiling) works through run_bass_kernel_spmd."""
    import sys
    import types

    try:
        from antenv.axon_hooks import get_axon_ntff_profile_hook  # noqa: F401
        return
    except ImportError:
        pass
    from trn_agent_boot.trn_boot import _ntff_profile_via_ctypes

    mod = types.ModuleType("antenv.axon_hooks")
    _hook = [_ntff_profile_via_ctypes("/opt/axon/libaxon_pjrt.so")]
    mod.get_axon_ntff_profile_hook = lambda: _hook[0]
    mod.set_axon_ntff_profile_hook = lambda h: _hook.__setitem__(0, h)
    sys.modules["antenv.axon_hooks"] = mod
    import antenv

    antenv.axon_hooks = mod


def _run(number_mask, reps, trace=False):
    from concourse.bass_utils import run_bass_kernel_spmd

    if trace:
        _install_ntff_hook()
    nc = _get_nc()
    in_maps = _shard_inputs(number_mask, reps)
    res = run_bass_kernel_spmd(
        nc, in_maps, core_ids=list(range(NCORES)), trace=trace
    )
    outs = np.stack([r["out"] for r in res.results], axis=0)
    return outs.reshape(B, M, D), res


def kernel(**inputs):
    out, _ = _run(inputs["number_mask"], inputs["reps"], trace=False)
    return out
